# revision 1
# baseline (speedup 1.0000x reference)
"""Trainium2 Bass kernel: MeshGraphNet-style GNN message passing.

Strategy (8 NeuronCores, SPMD):
  - Sort edges by dst. Partition nodes into 128-node blocks; assign a
    contiguous range of blocks to each core. Each core owns all edges whose
    dst falls in its blocks (contiguous in the sorted order).
  - Edge MLPs run feature-major on the PE array ([128 feat x edges]).
  - h[src] is fetched with indirect (gather) DMA from a replicated full
    node-state table, then PE-transposed to feature-major.
  - h[dst] needs no gather: edges are dst-sorted, so h[dst] for a strip is
    h_block^T @ O_T with a one-hot O_T generated on-chip.
  - scatter-mean: edges of one node block accumulate into a PSUM tile via
    one-hot matmuls (O[e, n] = (dst_local[e] == n)). The mean is a
    per-partition scale; the last-layer edge bias is a masked broadcast add.
  - Node-update MLP is per block. Updated node states are AllGathered
    between steps.

Register budgeting: every distinct dynamic-offset DMA expression permanently
consumes address registers on its issuing engine (only SP/Act/Pool can issue
DMAs, ~12 expressions max each). Per-block data is packed so each step loop
needs only 4 dynamic DMA expressions; per-block node states are fetched via
indirect DMA (register-free) using node ids stored in the metadata blob.
"""

import os
import numpy as np

P = 128
USE_F32R = True  # fast fp32 matmul mode for free-dim >= 256 matmuls

LAST = {}


def _ceil_div(a, b):
    return -(-a // b)


def _strips(T):
    out = []
    t0 = 0
    while t0 < T:
        k = min(4, T - t0)
        out.append((t0, k))
        t0 += k
    return out


def prep_host(inputs, n_cores=8):
    """Sort/pad/pack everything on the host."""
    x = np.asarray(inputs["x"], np.float32)
    ea = np.asarray(inputs["edge_attr"], np.float32)
    ei = np.asarray(inputs["edge_index"], np.int32)
    N, NI = x.shape
    E, EI = ea.shape
    L = np.asarray(inputs["ne_W1"]).shape[1]
    OD = np.asarray(inputs["de_W3"]).shape[1]
    S = np.asarray(inputs["pe_W1"]).shape[0]

    NB = _ceil_div(N, P)
    NB = _ceil_div(NB, n_cores) * n_cores
    BPC = NB // n_cores
    N_pad = NB * P

    src = ei[0].astype(np.int64)
    dst = ei[1].astype(np.int64)
    perm = np.argsort(dst, kind="stable")
    src_s = src[perm].astype(np.int32)
    dst_s = dst[perm].astype(np.int32)
    ea_s = ea[perm]

    deg = np.bincount(dst, minlength=N_pad).astype(np.float32)
    inv_deg = (1.0 / np.maximum(deg, 1.0)).astype(np.float32)
    mask = (deg > 0).astype(np.float32)

    block_start = np.searchsorted(dst_s, np.arange(0, N_pad + 1, P))
    cnt = np.diff(block_start)
    T = max(4, int(_ceil_div(cnt.max(), P)))
    E_blk = T * P

    # blob: per-node-block metadata, one row per (block, partition).
    # cols: 0=inv_deg, 1=mask, [2,2+T)=dloc by tile, 2+T=own node id
    # (i32 bits), [3+T,3+2T)=src ids (i32 bits). ownid adjoins src so one
    # indirect DMA gathers the own-block rows and all src rows together.
    C = 3 + 2 * T
    blob = np.zeros((NB, P, C), np.float32)
    blob[:, :, 0] = inv_deg.reshape(NB, P)
    blob[:, :, 1] = mask.reshape(NB, P)
    blob[:, :, 2 + T] = np.arange(N_pad, dtype=np.int32).reshape(
        NB, P).view(np.float32)
    blob[:, :, 2:2 + T] = -1.0
    dlocr = np.full((NB, E_blk), -1.0, np.float32)
    ea_pack = np.zeros((NB, E_blk, EI), np.float32)
    for b in range(NB):
        s0, s1 = int(block_start[b]), int(block_start[b + 1])
        n = s1 - s0
        if n == 0:
            continue
        sl = np.zeros(E_blk, np.int32)
        sl[:n] = src_s[s0:s1]
        blob[b, :, 3 + T:3 + 2 * T] = sl.reshape(T, P).T.view(np.float32)
        sf = np.full(E_blk, -1.0, np.float32)
        sf[:n] = (dst_s[s0:s1] - b * P).astype(np.float32)
        blob[b, :, 2:2 + T] = sf.reshape(T, P).T
        dlocr[b] = sf
        ea_pack[b, :n] = ea_s[s0:s1]

    x_fm = np.zeros((NI, N_pad), np.float32)
    x_fm[:, :N] = x.T

    params = dict(N=N, NI=NI, E=E, EI=EI, L=L, OD=OD, S=S,
                  NB=NB, BPC=BPC, N_pad=N_pad, T=T, E_blk=E_blk, C=C,
                  n_cores=n_cores)

    def wf(name):
        return np.ascontiguousarray(np.asarray(inputs[name], np.float32))

    weights = {
        "ne_W1": wf("ne_W1"), "ne_W2": wf("ne_W2"), "ne_W3": wf("ne_W3"),
        "ee_W1": wf("ee_W1"), "ee_W2": wf("ee_W2"), "ee_W3": wf("ee_W3"),
        "de_W1": wf("de_W1"), "de_W2": wf("de_W2"), "de_W3": wf("de_W3"),
        "pe_W1": wf("pe_W1").reshape(S * 3 * L, L),
        "pe_W2": wf("pe_W2").reshape(S * L, L),
        "pe_W3": wf("pe_W3").reshape(S * L, L),
        "pn_W1": wf("pn_W1").reshape(S * 2 * L, L),
        "pn_W2": wf("pn_W2").reshape(S * L, L),
        "pn_W3": wf("pn_W3").reshape(S * L, L),
        "ne_b1": wf("ne_b1").reshape(L, 1), "ne_b2": wf("ne_b2").reshape(L, 1),
        "ee_b1": wf("ee_b1").reshape(L, 1), "ee_b2": wf("ee_b2").reshape(L, 1),
        "ee_b3": wf("ee_b3").reshape(L, 1),
        "de_b1": wf("de_b1").reshape(L, 1), "de_b2": wf("de_b2").reshape(L, 1),
        "pe_b1": wf("pe_b1").reshape(S * L, 1),
        "pe_b2": wf("pe_b2").reshape(S * L, 1),
        "pn_b1": wf("pn_b1").reshape(S * L, 1),
        "pn_b2": wf("pn_b2").reshape(S * L, 1),
        "ne_b3": wf("ne_b3").reshape(1, L),
        "de_b3": wf("de_b3").reshape(1, OD),
        "pe_b3": wf("pe_b3").reshape(S, L),
        "pn_b3": wf("pn_b3").reshape(S, L),
    }

    in_maps = []
    for c in range(n_cores):
        b0, b1 = c * BPC, (c + 1) * BPC
        m = dict(weights)
        m["xfm"] = x_fm
        m["blob"] = np.ascontiguousarray(blob[b0:b1].reshape(BPC * P, C))
        m["dlocr"] = np.ascontiguousarray(dlocr[b0:b1])
        m["eafm"] = np.ascontiguousarray(
            ea_pack[b0:b1].reshape(BPC * E_blk, EI).T)
        in_maps.append(m)
    return params, in_maps


def build_program(params, debug=False):
    import concourse.bass as bass
    import concourse.bacc as bacc
    import concourse.mybir as mybir
    import concourse.tile as tile
    from concourse.bass import ds, ts
    from concourse.masks import make_identity
    from contextlib import ExitStack

    f32 = mybir.dt.float32
    f32r = mybir.dt.float32r
    i32 = mybir.dt.int32
    Relu = mybir.ActivationFunctionType.Relu
    AO = mybir.AluOpType

    NI, EI, L, OD, S = (params[k] for k in ("NI", "EI", "L", "OD", "S"))
    BPC, N_pad, T, E_blk, C = (params[k] for k in
                               ("BPC", "N_pad", "T", "E_blk", "C"))
    n_cores = params["n_cores"]
    E_cap = BPC * E_blk
    strips = _strips(T)

    fr = f32r if USE_F32R else f32

    def r(ap):  # kept for APs that are already rounded (no-op when disabled)
        return ap

    nc = bacc.Bacc(None, target_bir_lowering=False, debug=debug)

    def par(name, shape, dtype=f32, out=False):
        return nc.declare_dram_parameter(name, list(shape), dtype, isOutput=out)

    xfm_d = par("xfm", [NI, N_pad], fr)
    blob_d = par("blob", [BPC * P, C])
    dlocr_d = par("dlocr", [BPC, E_blk], fr)
    eafm_d = par("eafm", [EI, E_cap], fr)

    w_d = {}
    for nm, shp in [
        ("ne_W1", [NI, L]), ("ne_W2", [L, L]), ("ne_W3", [L, L]),
        ("ee_W1", [EI, L]), ("ee_W2", [L, L]), ("ee_W3", [L, L]),
        ("de_W1", [L, L]), ("de_W2", [L, L]), ("de_W3", [L, OD]),
        ("pe_W1", [S * 3 * L, L]), ("pe_W2", [S * L, L]), ("pe_W3", [S * L, L]),
        ("pn_W1", [S * 2 * L, L]), ("pn_W2", [S * L, L]), ("pn_W3", [S * L, L]),
        ("ne_b1", [L, 1]), ("ne_b2", [L, 1]),
        ("ee_b1", [L, 1]), ("ee_b2", [L, 1]), ("ee_b3", [L, 1]),
        ("de_b1", [L, 1]), ("de_b2", [L, 1]),
        ("pe_b1", [S * L, 1]), ("pe_b2", [S * L, 1]),
        ("pn_b1", [S * L, 1]), ("pn_b2", [S * L, 1]),
        ("ne_b3", [1, L]), ("de_b3", [1, OD]),
        ("pe_b3", [S, L]), ("pn_b3", [S, L]),
    ]:
        w_d[nm] = par(nm, shp)

    out_d = par("out", [BPC * P, OD], out=True)

    h_A = nc.dram_tensor("h_A", [N_pad, L], fr)
    h_B = nc.dram_tensor("h_B", [N_pad, L], fr, addr_space="Shared")
    h_C = nc.dram_tensor("h_C", [N_pad, L], fr, addr_space="Shared")
    h_own = nc.dram_tensor("h_own", [BPC * P, L], fr)
    eblk = nc.dram_tensor("eblk", [BPC * P, E_blk], fr)

    read_buf = [h_A, h_B, h_C, h_B, h_C]
    write_buf = [h_B, h_C, h_B, h_C, None]

    with tile.TileContext(nc) as tc, ExitStack() as ctx:
        wp = ctx.enter_context(tc.tile_pool(name="wp", bufs=1))
        sb2 = ctx.enter_context(tc.tile_pool(name="sb2", bufs=2))
        sb = ctx.enter_context(tc.tile_pool(name="sb", bufs=3))
        sbe = ctx.enter_context(tc.tile_pool(name="sbe", bufs=3))
        sbg = ctx.enter_context(tc.tile_pool(name="sbg", bufs=6))
        sbga = ctx.enter_context(tc.tile_pool(name="sbga", bufs=8))
        sbm = ctx.enter_context(tc.tile_pool(name="sbm", bufs=2))
        ps_s = ctx.enter_context(tc.tile_pool(name="ps_s", bufs=4, space="PSUM"))
        ps_b = ctx.enter_context(tc.tile_pool(name="ps_b", bufs=3, space="PSUM"))
        ps_a = ctx.enter_context(tc.tile_pool(name="ps_a", bufs=1, space="PSUM"))

        identity = wp.tile([P, P], f32, tag="identity")
        make_identity(nc, identity[:])
        iota_i = wp.tile([P, P], i32, tag="iota_i")
        nc.gpsimd.iota(iota_i[:], pattern=[[1, P]], base=0, channel_multiplier=0)
        iota_f = wp.tile([P, P], f32, tag="iota_f")
        nc.vector.tensor_copy(iota_f[:], iota_i[:])
        iotac_i = wp.tile([P, 1], i32, tag="iotac_i")
        nc.gpsimd.iota(iotac_i[:], pattern=[[1, 1]], base=0,
                       channel_multiplier=1)
        iotac_f = wp.tile([P, 1], f32, tag="iotac_f")
        nc.vector.tensor_copy(iotac_f[:], iotac_i[:])
        iotac_b = wp.tile([P, 512], f32, tag="iotac_b")
        nc.vector.tensor_copy(iotac_b[:],
                              iotac_f[:, :1].to_broadcast([P, 512])[:])
        ones_row = wp.tile([1, P], f32, tag="ones_row")
        nc.vector.memset(ones_row[:], 1.0)
        identity_r = wp.tile([P, P], fr, tag="identity_r")
        nc.vector.tensor_copy(identity_r[:], identity[:])
        ones_row_r = wp.tile([1, P], fr, tag="ones_row_r")
        nc.vector.tensor_copy(ones_row_r[:], ones_row[:])

        W = {}

        def load(nm, dram_ap, shape, tag, dt=f32):
            t = wp.tile(list(shape), dt, tag=tag)
            if dt is not f32:
                dram_ap = dram_ap.bitcast(dt)
            nc.sync.dma_start(out=t[:], in_=dram_ap)
            W[nm] = t
            return t

        load("ne_W1", w_d["ne_W1"][:, :], [NI, L], "ne_W1", fr)
        load("ne_W2", w_d["ne_W2"][:, :], [L, L], "ne_W2", fr)
        load("ne_W3", w_d["ne_W3"][:, :], [L, L], "ne_W3", fr)
        load("ee_W1", w_d["ee_W1"][:, :], [EI, L], "ee_W1", fr)
        load("ee_W2", w_d["ee_W2"][:, :], [L, L], "ee_W2", fr)
        load("ee_W3", w_d["ee_W3"][:, :], [L, L], "ee_W3", fr)
        load("de_W1", w_d["de_W1"][:, :], [L, L], "de_W1")
        load("de_W2", w_d["de_W2"][:, :], [L, L], "de_W2")
        load("de_W3", w_d["de_W3"][:, :], [L, OD], "de_W3")
        for nm in ("ne_b1", "ne_b2", "ee_b1", "ee_b2", "ee_b3", "de_b1",
                   "de_b2"):
            load(nm, w_d[nm][:, :], [L, 1], nm)
        load("ne_b3", w_d["ne_b3"][:, :], [1, L], "ne_b3")
        load("de_b3", w_d["de_b3"][:, :], [1, OD], "de_b3")
        for s in range(S):
            for k in range(3):
                load(f"pe_W1_{s}_{k}",
                     w_d["pe_W1"][s * 3 * L + k * L:s * 3 * L + (k + 1) * L, :],
                     [L, L], f"pe_W1_{s}_{k}", fr)
            for k in range(2):
                load(f"pn_W1_{s}_{k}",
                     w_d["pn_W1"][s * 2 * L + k * L:s * 2 * L + (k + 1) * L, :],
                     [L, L], f"pn_W1_{s}_{k}")
            for nm in ("pe_W2", "pe_W3"):
                load(f"{nm}_{s}", w_d[nm][s * L:(s + 1) * L, :], [L, L],
                     f"{nm}_{s}", fr)
            for nm in ("pn_W2", "pn_W3"):
                load(f"{nm}_{s}", w_d[nm][s * L:(s + 1) * L, :], [L, L],
                     f"{nm}_{s}")
            for nm in ("pe_b1", "pe_b2", "pn_b1", "pn_b2"):
                load(f"{nm}_{s}", w_d[nm][s * L:(s + 1) * L, :], [L, 1],
                     f"{nm}_{s}")
            for nm in ("pe_b3", "pn_b3"):
                load(f"{nm}_{s}", w_d[nm][s:s + 1, :], [1, L], f"{nm}_{s}")

        mm = nc.tensor.matmul

        # ---- node encoder: h_A for all nodes (identical on every core) ----
        NCHUNK = N_pad // 512
        with tc.For_i(0, NCHUNK, 1) as c:
            x_t = sb2.tile([NI, 512], fr, tag="x_t")
            nc.gpsimd.dma_start(out=x_t[:], in_=xfm_d[:, ts(c, 512)])
            p1 = ps_b.tile([P, 512], f32, tag="mm_big")
            mm(out=p1[:], lhsT=r(W["ne_W1"][:]), rhs=r(x_t[:]),
               start=True, stop=True)
            a1 = sb2.tile([P, 512], fr, tag="enc_a1")
            nc.scalar.activation(out=a1[:], in_=p1[:], func=Relu,
                                 bias=W["ne_b1"][:, :1])
            p2 = ps_b.tile([P, 512], f32, tag="mm_big")
            mm(out=p2[:], lhsT=r(W["ne_W2"][:]), rhs=r(a1[:]),
               start=True, stop=True)
            a2 = sb2.tile([P, 512], fr, tag="enc_a2")
            nc.scalar.activation(out=a2[:], in_=p2[:], func=Relu,
                                 bias=W["ne_b2"][:, :1])
            for j in range(4):
                p3 = ps_s.tile([P, L], f32, tag="mm_small")
                mm(out=p3[:], lhsT=a2[:, j * P:(j + 1) * P], rhs=W["ne_W3"][:],
                   start=True, stop=False)
                mm(out=p3[:], lhsT=ones_row[:], rhs=W["ne_b3"][:],
                   start=False, stop=True)
                h_sb = sb2.tile([P, L], fr, tag="enc_h")
                nc.vector.tensor_copy(h_sb[:], p3[:])
                nc.gpsimd.dma_start(out=h_A[ds(c * 512 + j * P, P), :],
                                    in_=h_sb[:])

        # ---- edge encoder -> eblk, one block per iteration ----
        with tc.For_i(0, BPC, 1) as b:
            ea_t = sbe.tile([P, E_blk], fr, tag="ebig")
            nc.gpsimd.dma_start(out=ea_t[:EI, :], in_=eafm_d[:, ts(b, E_blk)])
            e_all = sbe.tile([P, E_blk], fr, tag="ebig")
            for (t0, k) in strips:
                w = k * P
                cs = slice(t0 * P, t0 * P + w)
                p1 = ps_b.tile([P, w], f32, tag="mm_big")
                mm(out=p1[:], lhsT=r(W["ee_W1"][:]), rhs=r(ea_t[:EI, cs]),
                   start=True, stop=True)
                a1 = sb2.tile([P, w], fr, tag="ee_a1")
                nc.scalar.activation(out=a1[:], in_=p1[:], func=Relu,
                                     bias=W["ee_b1"][:, :1])
                p2 = ps_b.tile([P, w], f32, tag="mm_big")
                mm(out=p2[:], lhsT=r(W["ee_W2"][:]), rhs=r(a1[:]),
                   start=True, stop=True)
                a2 = sb2.tile([P, w], fr, tag="ee_a2")
                nc.scalar.activation(out=a2[:], in_=p2[:], func=Relu,
                                     bias=W["ee_b2"][:, :1])
                p3 = ps_b.tile([P, w], f32, tag="mm_big")
                mm(out=p3[:], lhsT=r(W["ee_W3"][:]), rhs=r(a2[:]),
                   start=True, stop=True)
                nc.vector.tensor_scalar_add(e_all[:, cs], p3[:],
                                            W["ee_b3"][:, :1])
            nc.gpsimd.dma_start(out=eblk[ts(b, P), :], in_=e_all[:])

        # ---- message passing steps ----
        for s in range(S):
            h_r = read_buf[s]
            b3p = ps_s.tile([P, L], f32, tag="mm_small")
            mm(out=b3p[:], lhsT=ones_row[:], rhs=W[f"pe_b3_{s}"][:],
               start=True, stop=True)
            b3b = wp.tile([P, L], f32, tag=f"b3b_{s}")
            nc.vector.tensor_copy(b3b[:], b3p[:])

            eng_be = nc.sync if s % 2 == 0 else nc.scalar
            eng_bo = nc.scalar if s % 2 == 0 else nc.sync
            with tc.For_i(0, BPC, 1) as b:
                blob_t = sbm.tile([P, C], f32, tag="blob_t")
                eng_be.dma_start(out=blob_t[:], in_=blob_d[ts(b, P), :])
                dlr_t = sbm.tile([1, E_blk], fr, tag="dlr_t")
                eng_be.dma_start(out=dlr_t[:], in_=dlocr_d[ds(b, 1), :])
                eb_t = sbe.tile([P, E_blk], fr, tag="ebig")
                eng_be.dma_start(out=eb_t[:], in_=eblk[ts(b, P), :])
                hblk_t = sb.tile([P, L], fr, tag="hblk")
                nc.gpsimd.indirect_dma_start(
                    out=hblk_t[:], out_offset=None, in_=h_r[:, :],
                    in_offset=bass.IndirectOffsetOnAxis(
                        ap=blob_t[:, 2 + T:3 + T].bitcast(i32), axis=0))
                hblk = hblk_t[:]

                agg_p = ps_a.tile([P, L], f32, tag="agg")
                first = True
                for (t0, k) in strips:
                    w = k * P
                    hs_fm = sb.tile([P, w], fr, tag="hs_fm")
                    for j in range(k):
                        t = t0 + j
                        g1 = sbga.tile([P, L], fr, tag="g_all")
                        nc.gpsimd.indirect_dma_start(
                            out=g1[:], out_offset=None, in_=h_r[:, :],
                            in_offset=bass.IndirectOffsetOnAxis(
                                ap=blob_t[:, 3 + T + t:4 + T + t].bitcast(i32),
                                axis=0))
                        tp1 = ps_s.tile([P, P], fr, tag="mm_small")
                        nc.tensor.transpose(out=tp1[:], in_=g1[:],
                                            identity=identity_r[:])
                        nc.vector.tensor_copy(hs_fm[:, j * P:(j + 1) * P],
                                              tp1[:])
                    # h[dst] via one-hot: O_T[n, e] = (dloc[e] == n)
                    dlb = ps_b.tile([P, w], f32, tag="mm_big")
                    mm(out=dlb[:], lhsT=ones_row_r[:],
                       rhs=dlr_t[:1, t0 * P:t0 * P + w], start=True, stop=True)
                    O_T = sb.tile([P, w], fr, tag="O_T")
                    nc.vector.tensor_tensor(out=O_T[:], in0=dlb[:],
                                            in1=iotac_b[:, :w], op=AO.is_equal)
                    hdp = ps_b.tile([P, w], f32, tag="mm_big")
                    mm(out=hdp[:], lhsT=r(hblk), rhs=r(O_T[:]),
                       start=True, stop=True)
                    hd_fm = sb.tile([P, w], fr, tag="hd_fm")
                    nc.scalar.activation(out=hd_fm[:], in_=hdp[:],
                                         func=mybir.ActivationFunctionType.Copy)

                    h1p = ps_b.tile([P, w], f32, tag="mm_big")
                    mm(out=h1p[:], lhsT=r(W[f"pe_W1_{s}_0"][:]),
                       rhs=r(hd_fm[:]), start=True, stop=False)
                    mm(out=h1p[:], lhsT=r(W[f"pe_W1_{s}_1"][:]),
                       rhs=r(hs_fm[:]), start=False, stop=False)
                    mm(out=h1p[:], lhsT=r(W[f"pe_W1_{s}_2"][:]),
                       rhs=r(eb_t[:, t0 * P:t0 * P + w]),
                       start=False, stop=True)
                    a1 = sb.tile([P, w], fr, tag="pe_a1")
                    nc.scalar.activation(out=a1[:], in_=h1p[:], func=Relu,
                                         bias=W[f"pe_b1_{s}"][:, :1])
                    h2p = ps_b.tile([P, w], f32, tag="mm_big")
                    mm(out=h2p[:], lhsT=r(W[f"pe_W2_{s}"][:]), rhs=r(a1[:]),
                       start=True, stop=True)
                    a2 = sb.tile([P, w], fr, tag="pe_a2")
                    nc.scalar.activation(out=a2[:], in_=h2p[:], func=Relu,
                                         bias=W[f"pe_b2_{s}"][:, :1])
                    for j in range(k):
                        t = t0 + j
                        mp = ps_s.tile([P, P], f32, tag="mm_small")
                        mm(out=mp[:], lhsT=a2[:, j * P:(j + 1) * P],
                           rhs=W[f"pe_W3_{s}"][:], start=True, stop=True)
                        msb = sbg.tile([P, P], f32, tag="msb")
                        nc.vector.tensor_copy(msb[:], mp[:])
                        O_j = sbg.tile([P, P], f32, tag="O_j")
                        nc.vector.tensor_tensor(
                            out=O_j[:],
                            in0=blob_t[:, 2 + t:3 + t].to_broadcast([P, P])[:],
                            in1=iota_f[:], op=AO.is_equal)
                        mm(out=agg_p[:], lhsT=O_j[:], rhs=msb[:],
                           start=first, stop=(t == T - 1))
                        first = False
                # agg = inv_deg * sum + mask * b3
                agg_sb = sb.tile([P, L], f32, tag="agg_sb")
                nc.vector.tensor_scalar(out=agg_sb[:], in0=agg_p[:],
                                        scalar1=blob_t[:, 0:1], scalar2=None,
                                        op0=AO.mult)
                b3m = sb.tile([P, L], f32, tag="b3m")
                nc.vector.tensor_scalar(out=b3m[:], in0=b3b[:],
                                        scalar1=blob_t[:, 1:2], scalar2=None,
                                        op0=AO.mult)
                nc.vector.tensor_tensor(out=agg_sb[:], in0=agg_sb[:],
                                        in1=b3m[:], op=AO.add)
                # node update MLP
                tr1 = ps_s.tile([P, P], fr, tag="mm_small")
                nc.tensor.transpose(out=tr1[:], in_=hblk,
                                    identity=identity_r[:])
                hfm = sb.tile([P, P], f32, tag="hfm")
                nc.vector.tensor_copy(hfm[:], tr1[:])
                tr2 = ps_s.tile([P, P], f32, tag="mm_small")
                nc.tensor.transpose(out=tr2[:], in_=agg_sb[:],
                                    identity=identity[:])
                afm = sb.tile([P, P], f32, tag="afm")
                nc.vector.tensor_copy(afm[:], tr2[:])
                n1p = ps_s.tile([P, P], f32, tag="mm_small")
                mm(out=n1p[:], lhsT=W[f"pn_W1_{s}_0"][:], rhs=hfm[:],
                   start=True, stop=False)
                mm(out=n1p[:], lhsT=W[f"pn_W1_{s}_1"][:], rhs=afm[:],
                   start=False, stop=True)
                n1 = sb.tile([P, P], f32, tag="n1")
                nc.scalar.activation(out=n1[:], in_=n1p[:], func=Relu,
                                     bias=W[f"pn_b1_{s}"][:, :1])
                n2p = ps_s.tile([P, P], f32, tag="mm_small")
                mm(out=n2p[:], lhsT=W[f"pn_W2_{s}"][:], rhs=n1[:],
                   start=True, stop=True)
                n2 = sb.tile([P, P], f32, tag="n2")
                nc.scalar.activation(out=n2[:], in_=n2p[:], func=Relu,
                                     bias=W[f"pn_b2_{s}"][:, :1])
                n3p = ps_s.tile([P, P], f32, tag="mm_small")
                mm(out=n3p[:], lhsT=n2[:], rhs=W[f"pn_W3_{s}"][:],
                   start=True, stop=False)
                mm(out=n3p[:], lhsT=ones_row[:], rhs=W[f"pn_b3_{s}"][:],
                   start=False, stop=True)
                hnew = sb.tile([P, L], fr, tag="hnew")
                nc.vector.tensor_tensor(out=hnew[:], in0=n3p[:], in1=hblk,
                                        op=AO.add)
                eng_bo.dma_start(out=h_own[ts(b, P), :], in_=hnew[:])
            if write_buf[s] is not None:
                nc.gpsimd.collective_compute(
                    "AllGather", mybir.AluOpType.bypass,
                    replica_groups=[list(range(n_cores))],
                    ins=[h_own[:, :]], outs=[write_buf[s][:, :]])

        # ---- decoder ----
        with tc.For_i(0, BPC, 1) as b:
            hblk = sb.tile([P, L], fr, tag="dec_hblk")
            nc.gpsimd.dma_start(out=hblk[:], in_=h_own[ts(b, P), :])
            tr = ps_s.tile([P, P], fr, tag="mm_small")
            nc.tensor.transpose(out=tr[:], in_=hblk[:],
                                identity=identity_r[:])
            hfm = sb.tile([P, P], f32, tag="dec_hfm")
            nc.vector.tensor_copy(hfm[:], tr[:])
            d1p = ps_s.tile([P, P], f32, tag="mm_small")
            mm(out=d1p[:], lhsT=W["de_W1"][:], rhs=hfm[:], start=True, stop=True)
            d1 = sb.tile([P, P], f32, tag="d1")
            nc.scalar.activation(out=d1[:], in_=d1p[:], func=Relu,
                                 bias=W["de_b1"][:, :1])
            d2p = ps_s.tile([P, P], f32, tag="mm_small")
            mm(out=d2p[:], lhsT=W["de_W2"][:], rhs=d1[:], start=True, stop=True)
            d2 = sb.tile([P, P], f32, tag="d2")
            nc.scalar.activation(out=d2[:], in_=d2p[:], func=Relu,
                                 bias=W["de_b2"][:, :1])
            dp = ps_s.tile([P, OD], f32, tag="mm_small")
            mm(out=dp[:], lhsT=d2[:], rhs=W["de_W3"][:], start=True, stop=False)
            mm(out=dp[:], lhsT=ones_row[:], rhs=W["de_b3"][:],
               start=False, stop=True)
            osb = sb.tile([P, OD], f32, tag="osb")
            nc.vector.tensor_copy(osb[:], dp[:])
            nc.gpsimd.dma_start(out=out_d[ts(b, P), :], in_=osb[:])

    nc.finalize()
    return nc


def _ensure_ntff_hook():
    """Register the axon NTFF profiling hook if the image lacks
    antenv.axon_hooks (replicates trn_boot's ctypes wiring)."""
    import sys
    import types
    try:
        import antenv.axon_hooks  # noqa: F401
        return
    except ImportError:
        pass
    import contextlib
    import ctypes
    import antenv

    m = types.ModuleType("antenv.axon_hooks")
    state = {"hook": None, "tried": False}

    def set_axon_ntff_profile_hook(hook):
        state["hook"] = hook

    def _make_hook(so_path="/opt/axon/libaxon_pjrt.so"):
        lib = ctypes.CDLL(so_path)
        if not hasattr(lib, "axon_start_nrt_profile"):
            return None
        lib.axon_start_nrt_profile.argtypes = [
            ctypes.POINTER(ctypes.c_int64), ctypes.c_size_t]
        lib.axon_start_nrt_profile.restype = ctypes.c_int64
        lib.axon_stop_nrt_profile.argtypes = [ctypes.c_char_p]
        lib.axon_stop_nrt_profile.restype = ctypes.c_int64

        @contextlib.contextmanager
        def _hook(output_dir, device_ids):
            import jax
            jax.devices()
            if device_ids:
                ids = (ctypes.c_int64 * len(device_ids))(*device_ids)
                rc = lib.axon_start_nrt_profile(ids, len(device_ids))
            else:
                rc = lib.axon_start_nrt_profile(None, 0)
            if rc != 0:
                raise RuntimeError(f"axon_start_nrt_profile rc={rc}")
            try:
                yield
            finally:
                n = lib.axon_stop_nrt_profile(str(output_dir).encode())
                print(f"ntff profile: {n} file(s) written to {output_dir}")

        return _hook

    def get_axon_ntff_profile_hook():
        if state["hook"] is None and not state["tried"]:
            state["tried"] = True
            try:
                state["hook"] = _make_hook()
            except OSError:
                state["hook"] = None
        return state["hook"]

    m.set_axon_ntff_profile_hook = set_axon_ntff_profile_hook
    m.get_axon_ntff_profile_hook = get_axon_ntff_profile_hook
    sys.modules["antenv.axon_hooks"] = m
    antenv.axon_hooks = m


def kernel(**inputs):
    n_cores = 8
    params, in_maps = prep_host(inputs, n_cores)
    nc = build_program(params, debug=False)

    from concourse.bass_utils import run_bass_kernel_spmd
    import time
    trace = bool(int(os.environ.get("KERNEL_TRACE", "0")))
    if trace:
        try:
            _ensure_ntff_hook()
        except Exception:
            pass
    t0 = time.time()
    try:
        res = run_bass_kernel_spmd(nc, in_maps, list(range(n_cores)),
                                   trace=trace)
    except ModuleNotFoundError:
        res = run_bass_kernel_spmd(nc, in_maps, list(range(n_cores)),
                                   trace=False)
    LAST["wall_s"] = time.time() - t0
    LAST["exec_time_ns"] = getattr(res, "exec_time_ns", None)
    LAST["profile_json"] = getattr(res, "profile_json", None)
    LAST["params"] = params
    out = np.concatenate([r["out"] for r in res.results], axis=0)
    return np.ascontiguousarray(out[:params["N"]].astype(np.float32))



# revision 18
# speedup vs baseline: 1.4929x; 1.4929x over previous
"""Trainium2 Bass kernel: MeshGraphNet-style GNN message passing (v2).

Strategy (8 NeuronCores, SPMD):
  - Sort edges by dst. 128-node blocks; core c owns blocks [c*BPC,(c+1)*BPC).
  - All matmul operands bf16 (PSUM f32): ~4-5x PE throughput vs fp32 and
    FWL weight loads.
  - h[src] gathered FEATURE-MAJOR in one shot via gpsimd.dma_gather
    (transpose=True, 256B bf16 rows): no per-tile indirect DMAs, no PE
    transposes.  int16 index limit (32767) handled by splitting each
    block's edges into src<32768 / src>=32768 segments, gathered from a
    base-offset view of the h table.  Uniform segment caps (CA/CB) across
    all blocks/cores keep the program SPMD.
  - h[dst] term of edge-MLP layer 1 fused: Zd = (W1d^T hblk^T) computed
    once per block node-major, then expanded per-edge inside the W1
    accumulation via the host-precomputed one-hot O_T (dloc==node).
  - scatter-mean via one-hot matmuls of the HIDDEN a2 (pre-W3) into a
    [128n,128f] PSUM accumulator C; W3 is applied once per block after
    the scatter (linear ops commute), with inv_deg folded into the C
    copy-out and b3 added as a rank-1 (b3 x mask) matmul.
  - Node-update MLP feature-major; residual added via identity-matmul
    accumulation; AllGather (bf16) of updated node states per step.
"""

import os
import numpy as np

P = 128
HALF = 32768
GATHER_MODE = os.environ.get("BASS_GATHER_MODE", "indirect1")

LAST = {}


def _ceil(a, b):
    return -(-a // b) * b


def _strips(T, w=4):
    out = []
    t0 = 0
    while t0 < T:
        k = min(w, T - t0)
        out.append((t0, k))
        t0 += k
    return out


def prep_host(inputs, n_cores=8):
    x = np.asarray(inputs["x"], np.float32)
    ea = np.asarray(inputs["edge_attr"], np.float32)
    ei = np.asarray(inputs["edge_index"], np.int32)
    N, NI = x.shape
    E, EI = ea.shape
    L = np.asarray(inputs["ne_W1"]).shape[1]
    OD = np.asarray(inputs["de_W3"]).shape[1]
    S = np.asarray(inputs["pe_W1"]).shape[0]

    NB = -(-N // P)
    NB = -(-NB // n_cores) * n_cores
    BPC = NB // n_cores
    N_pad = NB * P
    N_own = BPC * P

    src = ei[0].astype(np.int64)
    dst = ei[1].astype(np.int64)
    perm = np.argsort(dst, kind="stable")
    src_s = src[perm].astype(np.int32)
    dst_s = dst[perm].astype(np.int32)
    ea_s = ea[perm]

    deg = np.bincount(dst, minlength=N_pad).astype(np.float32)
    inv_deg = (1.0 / np.maximum(deg, 1.0)).astype(np.float32)
    mask = (deg > 0).astype(np.float32)

    block_start = np.searchsorted(dst_s, np.arange(0, N_pad + 1, P))

    # per-block segment split (src < HALF vs >=) and global uniform caps
    segs = []
    nA_max = nB_max = 0
    for b in range(NB):
        s0, s1 = int(block_start[b]), int(block_start[b + 1])
        sb = src_s[s0:s1]
        db = dst_s[s0:s1]
        eb = ea_s[s0:s1]
        lo = sb < HALF
        segs.append((sb[lo], db[lo], eb[lo], sb[~lo], db[~lo], eb[~lo]))
        nA_max = max(nA_max, int(lo.sum()))
        nB_max = max(nB_max, int((~lo).sum()))
    CA = max(_ceil(nA_max, P), P)
    CB = max(_ceil(nB_max, P), P)
    E_u = CA + CB
    T_u = E_u // P

    params = dict(N=N, NI=NI, E=E, EI=EI, L=L, OD=OD, S=S,
                  NB=NB, BPC=BPC, N_pad=N_pad, N_own=N_own,
                  CA=CA, CB=CB, E_u=E_u, T_u=T_u, n_cores=n_cores)

    import ml_dtypes

    def bf16_bits(a):
        return np.ascontiguousarray(
            np.asarray(a, np.float32).astype(ml_dtypes.bfloat16))

    wf = lambda n: np.asarray(inputs[n], np.float32)

    weights = {}
    # encoder/decoder weights
    for nm in ("ne_W1", "ne_W2", "ne_W3", "ee_W1", "ee_W2", "ee_W3",
               "de_W1", "de_W2", "de_W3"):
        weights[nm] = bf16_bits(wf(nm))
    # per-step W slices
    pe_W1 = wf("pe_W1")  # [S, 3L, L]
    pn_W1 = wf("pn_W1")  # [S, 2L, L]
    for s in range(S):
        weights[f"pe_W1d_{s}"] = bf16_bits(pe_W1[s, 0:L])
        weights[f"pe_W1s_{s}"] = bf16_bits(pe_W1[s, L:2 * L])
        weights[f"pe_W1e_{s}"] = bf16_bits(pe_W1[s, 2 * L:3 * L])
        weights[f"pn_W1h_{s}"] = bf16_bits(pn_W1[s, 0:L])
        weights[f"pn_W1a_{s}"] = bf16_bits(pn_W1[s, L:2 * L])
        weights[f"pe_W2_{s}"] = bf16_bits(wf("pe_W2")[s])
        weights[f"pe_W3_{s}"] = bf16_bits(wf("pe_W3")[s])
        weights[f"pn_W2_{s}"] = bf16_bits(wf("pn_W2")[s])
        weights[f"pn_W3_{s}"] = bf16_bits(wf("pn_W3")[s])
    # biases: column f32 for activation-bias; rows bf16 for rank-1 matmuls
    for nm in ("ne_b1", "ne_b2", "ee_b1", "ee_b2", "ee_b3", "de_b1", "de_b2"):
        weights[nm] = wf(nm).reshape(L, 1).astype(np.float32)
    weights["ne_b3"] = bf16_bits(wf("ne_b3").reshape(1, L))
    weights["de_b3"] = bf16_bits(wf("de_b3").reshape(1, OD))
    for s in range(S):
        for nm in ("pe_b1", "pe_b2", "pn_b1", "pn_b2"):
            weights[f"{nm}_{s}"] = wf(nm)[s].reshape(L, 1).astype(np.float32)
        weights[f"pe_b3_{s}"] = bf16_bits(wf("pe_b3")[s].reshape(1, L))
        weights[f"pn_b3_{s}"] = bf16_bits(wf("pn_b3")[s].reshape(1, L))

    in_maps = []
    for c in range(n_cores):
        b0 = c * BPC
        eafm = np.zeros((EI, BPC * E_u), np.float32)
        OT = np.zeros((BPC * P, E_u), np.float32)
        idx16 = np.zeros((BPC * P, E_u // 16), np.int16)
        idx32 = np.zeros((BPC * P, T_u), np.int32)
        dlocs = np.full((P, BPC * T_u), -1.0, np.float32)
        invd = np.zeros((P, BPC), np.float32)
        maskr = np.zeros((1, BPC * P), np.float32)
        for j in range(BPC):
            b = b0 + j
            sA, dA, eA, sB, dB, eB = segs[b]
            nA, nB_ = len(sA), len(sB)
            # column layout: [segA | padA | segB | padB]
            cols_src = np.zeros(E_u, np.int32)
            cols_src[:nA] = sA
            cols_src[CA:CA + nB_] = sB - HALF
            dloc = np.full(E_u, -1, np.int32)
            dloc[:nA] = dA - b * P
            dloc[CA:CA + nB_] = dB - b * P
            eac = np.zeros((E_u, EI), np.float32)
            eac[:nA] = eA
            eac[CA:CA + nB_] = eB
            eafm[:, j * E_u:(j + 1) * E_u] = eac.T
            valid = dloc >= 0
            OT[j * P:(j + 1) * P, :] = (
                dloc[None, :] == np.arange(P)[:, None]) & valid[None, :]
            ii = np.arange(E_u)
            idx16[j * P + (ii % 16), ii // 16] = cols_src.astype(np.int16)
            gsrc = cols_src.copy()
            gsrc[CA:] += HALF
            idx32[j * P:(j + 1) * P, :] = gsrc.reshape(T_u, P).T
            dlocs[:, j * T_u:(j + 1) * T_u] = dloc.reshape(T_u, P).T
            invd[:, j] = inv_deg[b * P:(b + 1) * P]
            maskr[0, j * P:(j + 1) * P] = mask[b * P:(b + 1) * P]
        x_own = np.zeros((NI, N_own), np.float32)
        nreal = min(N_own, max(0, N - b0 * P))
        x_own[:, :nreal] = x.T[:, b0 * P:b0 * P + nreal]

        m = dict(weights)
        m["eafm"] = bf16_bits(eafm)
        m["OT"] = bf16_bits(OT)
        m["idx"] = idx16
        m["idx32"] = idx32
        m["dlocs"] = np.ascontiguousarray(dlocs)
        m["invd"] = np.ascontiguousarray(invd)
        m["maskr"] = bf16_bits(maskr)
        m["xfm"] = bf16_bits(x_own)
        in_maps.append(m)
    return params, in_maps


def build_program(params, debug=False):
    import concourse.bass as bass
    import concourse.bacc as bacc
    import concourse.mybir as mybir
    import concourse.tile as tile
    from concourse.bass import ds, ts
    from concourse.masks import make_identity
    from contextlib import ExitStack

    f32 = mybir.dt.float32
    bf16 = mybir.dt.bfloat16
    i16 = mybir.dt.int16
    i32 = mybir.dt.int32
    Relu = mybir.ActivationFunctionType.Relu
    AO = mybir.AluOpType

    NI, EI, L, OD, S = (params[k] for k in ("NI", "EI", "L", "OD", "S"))
    BPC, N_pad, N_own = (params[k] for k in ("BPC", "N_pad", "N_own"))
    CA, CB, E_u, T_u = (params[k] for k in ("CA", "CB", "E_u", "T_u"))
    n_cores = params["n_cores"]
    strips = _strips(T_u)

    nc = bacc.Bacc(None, target_bir_lowering=False, debug=debug)

    def par(name, shape, dtype=f32, out=False):
        return nc.declare_dram_parameter(name, list(shape), dtype, isOutput=out)

    eafm_d = par("eafm", [EI, BPC * E_u], bf16)
    OT_d = par("OT", [BPC * P, E_u], bf16)
    idx_d = par("idx", [BPC * P, E_u // 16], i16)
    idx32_d = par("idx32", [BPC * P, T_u], i32)
    dlocs_d = par("dlocs", [P, BPC * T_u])
    invd_d = par("invd", [P, BPC])
    maskr_d = par("maskr", [1, BPC * P], bf16)
    xfm_d = par("xfm", [NI, N_own], bf16)

    w_d = {}
    wshapes = {"ne_W1": [NI, L], "ne_W2": [L, L], "ne_W3": [L, L],
               "ee_W1": [EI, L], "ee_W2": [L, L], "ee_W3": [L, L],
               "de_W1": [L, L], "de_W2": [L, L], "de_W3": [L, OD]}
    for s in range(S):
        for nm in ("pe_W1d", "pe_W1s", "pe_W1e", "pe_W2", "pe_W3",
                   "pn_W1h", "pn_W1a", "pn_W2", "pn_W3"):
            wshapes[f"{nm}_{s}"] = [L, L]
    for nm, shp in wshapes.items():
        w_d[nm] = par(nm, shp, bf16)
    bshapes = {"ne_b1": [L, 1], "ne_b2": [L, 1], "ee_b1": [L, 1],
               "ee_b2": [L, 1], "ee_b3": [L, 1], "de_b1": [L, 1],
               "de_b2": [L, 1]}
    for s in range(S):
        for nm in ("pe_b1", "pe_b2", "pn_b1", "pn_b2"):
            bshapes[f"{nm}_{s}"] = [L, 1]
    for nm, shp in bshapes.items():
        w_d[nm] = par(nm, shp, f32)
    rshapes = {"ne_b3": [1, L], "de_b3": [1, OD]}
    for s in range(S):
        rshapes[f"pe_b3_{s}"] = [1, L]
        rshapes[f"pn_b3_{s}"] = [1, L]
    for nm, shp in rshapes.items():
        w_d[nm] = par(nm, shp, bf16)

    out_d = par("out", [N_own, OD], out=True)

    h_ownA = nc.dram_tensor("h_ownA", [N_own, L], bf16)
    h_ownB = nc.dram_tensor("h_ownB", [N_own, L], bf16)
    h_bufs = [nc.dram_tensor(f"h_nm{i}", [N_pad, L], bf16,
                             addr_space="Shared") for i in range(3)]
    eblk = nc.dram_tensor("eblk", [BPC * P, E_u], bf16)

    with tile.TileContext(nc) as tc, ExitStack() as ctx:
        wp = ctx.enter_context(tc.tile_pool(name="wp", bufs=1))
        sbx = ctx.enter_context(tc.tile_pool(name="sbx", bufs=3))
        sbe = ctx.enter_context(tc.tile_pool(name="sbe", bufs=3))
        sbo = ctx.enter_context(tc.tile_pool(name="sbo", bufs=3))
        sbh = ctx.enter_context(tc.tile_pool(name="sbh", bufs=3))
        sbg = ctx.enter_context(tc.tile_pool(name="sbg", bufs=2))
        sbi = ctx.enter_context(tc.tile_pool(name="sbi", bufs=3))
        sbs = ctx.enter_context(tc.tile_pool(name="sbs", bufs=4))
        sbm = ctx.enter_context(tc.tile_pool(name="sbm", bufs=8))
        ps_b = ctx.enter_context(tc.tile_pool(name="ps_b", bufs=3, space="PSUM"))
        ps_s = ctx.enter_context(tc.tile_pool(name="ps_s", bufs=2, space="PSUM"))
        ps_t = ctx.enter_context(tc.tile_pool(name="ps_t", bufs=2, space="PSUM"))
        ps_c = ctx.enter_context(tc.tile_pool(name="ps_c", bufs=1, space="PSUM"))

        identity_f = wp.tile([P, P], f32, tag="identity_f")
        make_identity(nc, identity_f[:])
        identity = wp.tile([P, P], bf16, tag="identity")
        nc.vector.tensor_copy(identity[:], identity_f[:])
        iota_i = wp.tile([P, P], i32, tag="iota_i")
        nc.gpsimd.iota(iota_i[:], pattern=[[1, P]], base=0, channel_multiplier=0)
        iota_f = wp.tile([P, P], f32, tag="iota_f")
        nc.vector.tensor_copy(iota_f[:], iota_i[:])
        ones_row = wp.tile([1, P], bf16, tag="ones_row")
        nc.vector.memset(ones_row[:], 1.0)

        W = {}

        def load(nm, dt, shape=None):
            shp = shape or wshapes.get(nm) or bshapes.get(nm) or rshapes[nm]
            t = wp.tile(list(shp), dt, tag=f"w_{nm}")
            nc.sync.dma_start(out=t[:], in_=w_d[nm][:, :])
            W[nm] = t

        for nm in wshapes:
            load(nm, bf16)
        for nm in bshapes:
            load(nm, f32)
        for nm in rshapes:
            load(nm, bf16)

        dlocs_t = wp.tile([P, BPC * T_u], f32, tag="dlocs_t")
        nc.sync.dma_start(out=dlocs_t[:], in_=dlocs_d[:, :])
        invd_t = wp.tile([P, BPC], f32, tag="invd_t")
        nc.sync.dma_start(out=invd_t[:], in_=invd_d[:, :])
        maskr_t = wp.tile([1, BPC * P], bf16, tag="maskr_t")
        nc.sync.dma_start(out=maskr_t[:], in_=maskr_d[:, :])

        mm = nc.tensor.matmul

        # ---- node encoder (own shard), 128-node chunks ----
        with tc.For_i(0, BPC, 1) as cch:
            x_t = sbx.tile([NI, P], bf16, tag="x_t")
            nc.sync.dma_start(out=x_t[:], in_=xfm_d[:, ts(cch, P)])
            p1 = ps_s.tile([P, P], f32, tag="mm_small")
            mm(out=p1[:], lhsT=W["ne_W1"][:], rhs=x_t[:], start=True, stop=True)
            a1 = sbm.tile([P, P], bf16, tag="enc_a1")
            nc.scalar.activation(out=a1[:], in_=p1[:], func=Relu,
                                 bias=W["ne_b1"][:, :1])
            p2 = ps_s.tile([P, P], f32, tag="mm_small")
            mm(out=p2[:], lhsT=W["ne_W2"][:], rhs=a1[:], start=True, stop=True)
            a2 = sbm.tile([P, P], bf16, tag="enc_a2")
            nc.scalar.activation(out=a2[:], in_=p2[:], func=Relu,
                                 bias=W["ne_b2"][:, :1])
            p3 = ps_s.tile([P, L], f32, tag="mm_small")
            mm(out=p3[:], lhsT=a2[:], rhs=W["ne_W3"][:],
               start=True, stop=False)
            mm(out=p3[:], lhsT=ones_row[:], rhs=W["ne_b3"][:],
               start=False, stop=True)
            h_sb = sbm.tile([P, L], bf16, tag="enc_h")
            nc.vector.tensor_copy(h_sb[:], p3[:])
            nc.gpsimd.dma_start(out=h_ownA[ts(cch, P), :], in_=h_sb[:])

        # ---- edge encoder ----
        with tc.For_i(0, BPC, 1) as b:
            ea_t = sbe.tile([EI, E_u], bf16, tag="ea_t")
            nc.scalar.dma_start(out=ea_t[:], in_=eafm_d[:, ts(b, E_u)])
            e_all = sbe.tile([P, E_u], bf16, tag="e_all")
            for (t0, k) in strips:
                w = k * P
                cs = slice(t0 * P, t0 * P + w)
                p1 = ps_b.tile([P, 512], f32, tag="mm_big")
                mm(out=p1[:, :w], lhsT=W["ee_W1"][:], rhs=ea_t[:, cs],
                   start=True, stop=True)
                a1 = sbs.tile([P, 512], bf16, tag="ee_a1")
                nc.scalar.activation(out=a1[:, :w], in_=p1[:, :w], func=Relu,
                                     bias=W["ee_b1"][:, :1])
                p2 = ps_b.tile([P, 512], f32, tag="mm_big")
                mm(out=p2[:, :w], lhsT=W["ee_W2"][:], rhs=a1[:, :w],
                   start=True, stop=True)
                a2 = sbs.tile([P, 512], bf16, tag="ee_a2")
                nc.scalar.activation(out=a2[:, :w], in_=p2[:, :w], func=Relu,
                                     bias=W["ee_b2"][:, :1])
                p3 = ps_b.tile([P, 512], f32, tag="mm_big")
                mm(out=p3[:, :w], lhsT=W["ee_W3"][:], rhs=a2[:, :w],
                   start=True, stop=True)
                nc.vector.tensor_scalar_add(e_all[:, cs], p3[:, :w],
                                            W["ee_b3"][:, :1])
            nc.gpsimd.dma_start(out=eblk[ts(b, P), :], in_=e_all[:])

        nc.gpsimd.collective_compute(
            "AllGather", mybir.AluOpType.bypass,
            replica_groups=[list(range(n_cores))],
            ins=[h_ownA[:, :]], outs=[h_bufs[0][:, :]])

        # ---- message-passing steps ----
        for s in range(S):
            h_r = h_bufs[s % 3]
            h_w = h_bufs[(s + 1) % 3]
            ho_r = h_ownA if s % 2 == 0 else h_ownB
            ho_w = h_ownB if s % 2 == 0 else h_ownA
            eng_a = nc.sync if s % 2 == 0 else nc.scalar
            eng_b = nc.scalar if s % 2 == 0 else nc.sync
            with tc.For_i(0, BPC, 1) as b:
                idx_t = sbi.tile([P, E_u // 16], i16, tag="idx_t")
                eng_b.dma_start(out=idx_t[:], in_=idx_d[ts(b, P), :])
                eb_t = sbe.tile([P, E_u], bf16, tag="eb_t")
                eng_a.dma_start(out=eb_t[:], in_=eblk[ts(b, P), :])
                ot_t = sbo.tile([P, E_u], bf16, tag="ot_t")
                eng_a.dma_start(out=ot_t[:], in_=OT_d[ts(b, P), :])
                hblk = sbm.tile([P, L], bf16, tag="hblk")
                eng_b.dma_start(out=hblk[:], in_=ho_r[ts(b, P), :])

                hs_t = sbh.tile([P, 1, E_u], bf16, tag="hs_t")
                if GATHER_MODE == "dma_gather":
                    nc.gpsimd.dma_gather(
                        hs_t[:, :, 0:CA], h_r[0:HALF, :],
                        idx_t[:, 0:CA // 16], CA, CA, L, transpose=True)
                    nc.gpsimd.dma_gather(
                        hs_t[:, :, CA:E_u], h_r[HALF:N_pad, :],
                        idx_t[:, CA // 16:E_u // 16],
                        CB, CB, L, transpose=True)
                elif GATHER_MODE == "indirect1":
                    idx32_t = sbi.tile([P, T_u], i32, tag="idx32_t")
                    eng_b.dma_start(out=idx32_t[:], in_=idx32_d[ts(b, P), :])
                    g_nm = sbg.tile([P, T_u * L], bf16, tag="g_nm1")
                    nc.gpsimd.indirect_dma_start(
                        out=g_nm[:], out_offset=None, in_=h_r[:, :],
                        in_offset=bass.IndirectOffsetOnAxis(
                            ap=idx32_t[:, 0:T_u], axis=0))
                    for t in range(T_u):
                        gtp = ps_t.tile([P, P], bf16, tag="mm_tr")
                        nc.tensor.transpose(out=gtp[:],
                                            in_=g_nm[:, t * L:(t + 1) * L],
                                            identity=identity[:])
                        nc.vector.tensor_copy(hs_t[:, 0, t * P:(t + 1) * P],
                                              gtp[:])
                else:
                    idx32_t = sbi.tile([P, T_u], i32, tag="idx32_t")
                    eng_b.dma_start(out=idx32_t[:], in_=idx32_d[ts(b, P), :])
                    for t in range(T_u):
                        g1 = sbm.tile([P, L], bf16, tag="g_nm")
                        nc.gpsimd.indirect_dma_start(
                            out=g1[:], out_offset=None, in_=h_r[:, :],
                            in_offset=bass.IndirectOffsetOnAxis(
                                ap=idx32_t[:, t:t + 1], axis=0))
                        gtp = ps_t.tile([P, P], bf16, tag="mm_tr")
                        nc.tensor.transpose(out=gtp[:], in_=g1[:],
                                            identity=identity[:])
                        nc.vector.tensor_copy(hs_t[:, 0, t * P:(t + 1) * P],
                                              gtp[:])

                # hblk_fm = hblk^T ; Zd = hblk @ W1d (node-major)
                trp = ps_t.tile([P, P], bf16, tag="mm_tr")
                nc.tensor.transpose(out=trp[:], in_=hblk[:],
                                    identity=identity[:])
                hblk_fm = sbm.tile([P, P], bf16, tag="hblk_fm")
                nc.vector.tensor_copy(hblk_fm[:], trp[:])
                zdp = ps_s.tile([P, P], f32, tag="mm_small")
                mm(out=zdp[:], lhsT=hblk_fm[:], rhs=W[f"pe_W1d_{s}"][:],
                   start=True, stop=True)
                zd = sbm.tile([P, P], bf16, tag="zd")
                nc.vector.tensor_copy(zd[:], zdp[:])

                # edge MLP + scatter of a2 into C
                cp = ps_c.tile([P, P], f32, tag="c_acc")
                first = True
                for (t0, k) in strips:
                    w = k * P
                    cs = slice(t0 * P, t0 * P + w)
                    p1 = ps_b.tile([P, 512], f32, tag="mm_big")
                    mm(out=p1[:, :w], lhsT=zd[:], rhs=ot_t[:, cs],
                       start=True, stop=False)
                    mm(out=p1[:, :w], lhsT=W[f"pe_W1s_{s}"][:],
                       rhs=hs_t[:, 0, cs], start=False, stop=False)
                    mm(out=p1[:, :w], lhsT=W[f"pe_W1e_{s}"][:],
                       rhs=eb_t[:, cs], start=False, stop=True)
                    a1 = sbs.tile([P, 512], bf16, tag="pe_a1")
                    nc.scalar.activation(out=a1[:, :w], in_=p1[:, :w],
                                         func=Relu, bias=W[f"pe_b1_{s}"][:, :1])
                    p2 = ps_b.tile([P, 512], f32, tag="mm_big")
                    mm(out=p2[:, :w], lhsT=W[f"pe_W2_{s}"][:], rhs=a1[:, :w],
                       start=True, stop=True)
                    a2 = sbs.tile([P, 512], bf16, tag="pe_a2")
                    nc.scalar.activation(out=a2[:, :w], in_=p2[:, :w],
                                         func=Relu, bias=W[f"pe_b2_{s}"][:, :1])
                    for j in range(k):
                        t = t0 + j
                        tp = ps_t.tile([P, P], bf16, tag="mm_tr")
                        nc.tensor.transpose(out=tp[:],
                                            in_=a2[:, j * P:(j + 1) * P],
                                            identity=identity[:])
                        a2e = sbm.tile([P, P], bf16, tag="a2e")
                        nc.vector.tensor_copy(a2e[:], tp[:])
                        oj = sbm.tile([P, P], bf16, tag="oj")
                        nc.vector.tensor_scalar(
                            out=oj[:], in0=iota_f[:],
                            scalar1=dlocs_t[:, ds(b * T_u + t, 1)],
                            scalar2=None, op0=AO.is_equal)
                        mm(out=cp[:], lhsT=oj[:], rhs=a2e[:],
                           start=first, stop=(t == T_u - 1))
                        first = False
                # C -> inv_deg scale -> transpose -> W3 + b3 x mask
                c_sb = sbm.tile([P, P], bf16, tag="c_sb")
                nc.vector.tensor_scalar(out=c_sb[:], in0=cp[:],
                                        scalar1=invd_t[:, ds(b, 1)],
                                        scalar2=None, op0=AO.mult)
                ctp = ps_t.tile([P, P], bf16, tag="mm_tr")
                nc.tensor.transpose(out=ctp[:], in_=c_sb[:],
                                    identity=identity[:])
                c_fm = sbm.tile([P, P], bf16, tag="c_fm")
                nc.vector.tensor_copy(c_fm[:], ctp[:])
                aggp = ps_s.tile([P, P], f32, tag="mm_small")
                mm(out=aggp[:], lhsT=W[f"pe_W3_{s}"][:], rhs=c_fm[:],
                   start=True, stop=False)
                mm(out=aggp[:], lhsT=W[f"pe_b3_{s}"][:],
                   rhs=maskr_t[:, ts(b, P)], start=False, stop=True)
                agg_fm = sbm.tile([P, P], bf16, tag="agg_fm")
                nc.vector.tensor_copy(agg_fm[:], aggp[:])

                # node-update MLP (feature-major)
                n1p = ps_s.tile([P, P], f32, tag="mm_small")
                mm(out=n1p[:], lhsT=W[f"pn_W1h_{s}"][:], rhs=hblk_fm[:],
                   start=True, stop=False)
                mm(out=n1p[:], lhsT=W[f"pn_W1a_{s}"][:], rhs=agg_fm[:],
                   start=False, stop=True)
                n1 = sbm.tile([P, P], bf16, tag="n1")
                nc.scalar.activation(out=n1[:], in_=n1p[:], func=Relu,
                                     bias=W[f"pn_b1_{s}"][:, :1])
                n2p = ps_s.tile([P, P], f32, tag="mm_small")
                mm(out=n2p[:], lhsT=W[f"pn_W2_{s}"][:], rhs=n1[:],
                   start=True, stop=True)
                n2 = sbm.tile([P, P], bf16, tag="n2")
                nc.scalar.activation(out=n2[:], in_=n2p[:], func=Relu,
                                     bias=W[f"pn_b2_{s}"][:, :1])
                n3p = ps_s.tile([P, P], f32, tag="mm_small")
                mm(out=n3p[:], lhsT=n2[:], rhs=W[f"pn_W3_{s}"][:],
                   start=True, stop=False)
                mm(out=n3p[:], lhsT=ones_row[:], rhs=W[f"pn_b3_{s}"][:],
                   start=False, stop=False)
                mm(out=n3p[:], lhsT=identity[:], rhs=hblk[:],
                   start=False, stop=True)
                hnew = sbm.tile([P, L], bf16, tag="hnew")
                nc.vector.tensor_copy(hnew[:], n3p[:])
                nc.gpsimd.dma_start(out=ho_w[ts(b, P), :], in_=hnew[:])
            if s < S - 1:
                nc.gpsimd.collective_compute(
                    "AllGather", mybir.AluOpType.bypass,
                    replica_groups=[list(range(n_cores))],
                    ins=[ho_w[:, :]], outs=[h_w[:, :]])

        # ---- decoder ----
        h_fin = h_ownB if S % 2 == 1 else h_ownA
        with tc.For_i(0, BPC, 1) as b:
            hblk = sbm.tile([P, L], bf16, tag="dec_hblk")
            nc.gpsimd.dma_start(out=hblk[:], in_=h_fin[ts(b, P), :])
            trp = ps_t.tile([P, P], bf16, tag="mm_tr")
            nc.tensor.transpose(out=trp[:], in_=hblk[:], identity=identity[:])
            hfm = sbm.tile([P, P], bf16, tag="dec_hfm")
            nc.vector.tensor_copy(hfm[:], trp[:])
            d1p = ps_s.tile([P, P], f32, tag="mm_small")
            mm(out=d1p[:], lhsT=W["de_W1"][:], rhs=hfm[:], start=True, stop=True)
            d1 = sbm.tile([P, P], bf16, tag="d1")
            nc.scalar.activation(out=d1[:], in_=d1p[:], func=Relu,
                                 bias=W["de_b1"][:, :1])
            d2p = ps_s.tile([P, P], f32, tag="mm_small")
            mm(out=d2p[:], lhsT=W["de_W2"][:], rhs=d1[:], start=True, stop=True)
            d2 = sbm.tile([P, P], bf16, tag="d2")
            nc.scalar.activation(out=d2[:], in_=d2p[:], func=Relu,
                                 bias=W["de_b2"][:, :1])
            dp = ps_s.tile([P, OD], f32, tag="mm_small")
            mm(out=dp[:], lhsT=d2[:], rhs=W["de_W3"][:], start=True, stop=False)
            mm(out=dp[:], lhsT=ones_row[:], rhs=W["de_b3"][:],
               start=False, stop=True)
            osb = sbm.tile([P, OD], f32, tag="osb")
            nc.vector.tensor_copy(osb[:], dp[:])
            nc.gpsimd.dma_start(out=out_d[ts(b, P), :], in_=osb[:])

    nc.finalize()
    return nc


def _ensure_ntff_hook():
    """Register the axon NTFF profiling hook if the image lacks
    antenv.axon_hooks (replicates trn_boot's ctypes wiring)."""
    import sys
    import types
    try:
        import antenv.axon_hooks  # noqa: F401
        return
    except ImportError:
        pass
    import contextlib
    import ctypes
    import antenv

    m = types.ModuleType("antenv.axon_hooks")
    state = {"hook": None, "tried": False}

    def set_axon_ntff_profile_hook(hook):
        state["hook"] = hook

    def _make_hook(so_path="/opt/axon/libaxon_pjrt.so"):
        lib = ctypes.CDLL(so_path)
        if not hasattr(lib, "axon_start_nrt_profile"):
            return None
        lib.axon_start_nrt_profile.argtypes = [
            ctypes.POINTER(ctypes.c_int64), ctypes.c_size_t]
        lib.axon_start_nrt_profile.restype = ctypes.c_int64
        lib.axon_stop_nrt_profile.argtypes = [ctypes.c_char_p]
        lib.axon_stop_nrt_profile.restype = ctypes.c_int64

        @contextlib.contextmanager
        def _hook(output_dir, device_ids):
            import jax
            jax.devices()
            if device_ids:
                ids = (ctypes.c_int64 * len(device_ids))(*device_ids)
                rc = lib.axon_start_nrt_profile(ids, len(device_ids))
            else:
                rc = lib.axon_start_nrt_profile(None, 0)
            if rc != 0:
                raise RuntimeError(f"axon_start_nrt_profile rc={rc}")
            try:
                yield
            finally:
                n = lib.axon_stop_nrt_profile(str(output_dir).encode())
                print(f"ntff profile: {n} file(s) written to {output_dir}")

        return _hook

    def get_axon_ntff_profile_hook():
        if state["hook"] is None and not state["tried"]:
            state["tried"] = True
            try:
                state["hook"] = _make_hook()
            except OSError:
                state["hook"] = None
        return state["hook"]

    m.set_axon_ntff_profile_hook = set_axon_ntff_profile_hook
    m.get_axon_ntff_profile_hook = get_axon_ntff_profile_hook
    sys.modules["antenv.axon_hooks"] = m
    antenv.axon_hooks = m


def kernel(**inputs):
    n_cores = 8
    params, in_maps = prep_host(inputs, n_cores)
    nc = build_program(params, debug=False)

    from concourse.bass_utils import run_bass_kernel_spmd
    import time
    trace = bool(int(os.environ.get("KERNEL_TRACE", "0")))
    if trace:
        try:
            _ensure_ntff_hook()
        except Exception:
            pass
    t0 = time.time()
    try:
        res = run_bass_kernel_spmd(nc, in_maps, list(range(n_cores)),
                                   trace=trace)
    except ModuleNotFoundError:
        res = run_bass_kernel_spmd(nc, in_maps, list(range(n_cores)),
                                   trace=False)
    LAST["wall_s"] = time.time() - t0
    LAST["exec_time_ns"] = getattr(res, "exec_time_ns", None)
    LAST["profile_json"] = getattr(res, "profile_json", None)
    LAST["params"] = params
    out = np.concatenate([r["out"] for r in res.results], axis=0)
    return np.ascontiguousarray(out[:params["N"]].astype(np.float32))
